# revision 47
# baseline (speedup 1.0000x reference)
"""Trainium2 Bass kernel for nn_DTHyperNet (soft decision tree hypernetwork).

Contract: kernel(**inputs) takes the FULL unsharded inputs (B=8192) as
numpy arrays and returns the FULL [8192, 100] float32 output. Internally
the batch is sharded 8 ways (pure data parallel, weights replicated) and
one Bass/Tile program is compiled and run SPMD on NeuronCores 0-7.

Math (eval mode):
  trunk:  h = relu(bn(x @ w_in + b_in))  [+ residual gelu blocks, which
          collapse to identity when bn2 weight/bias are zero - detected
          from the actual input values and skipped]
  heads:  fi/fs = h @ w_fi/w_fs  (15 nodes x 512 features)
          lnc   = h @ w_lc       (16 leaves x 100 classes)
  per node: sd = sigmoid(sum_f softmax(fi)*(x - fs))
  routing coeff[leaf] = prod_d (sd or 1-sd) along the tree path
  out = sum_l coeff_l * lnc_l

Implementation notes:
  - All matmul contractions run with the contraction dim on SBUF
    partitions; the trunk is computed directly in transposed layout
    (hT: [H partitions, batch free]) so no on-device transposes needed.
  - fi/fs head matmuls run in bf16 (their rounding errors cancel in the
    softmax ratio num/den); trunk and leaf-logit matmuls run in float32r
    (rounded fp32, 2 cycles/row on the PE, ~1.5e-4 matmul accuracy).
  - softmax-dot is fused: ACT computes P=exp(fi) with a free running-sum
    (den); DVE computes (x-fs) and P*(x-fs) with a fused accumulate (num)
    - two DVE instructions per (node, batch-tile).
  - This walrus build rejects instructions with more than one semaphore
    wait, so a post-pass splits multi-wait instructions by hoisting
    excess waits onto same-engine NOPs.
"""
import os
import sys
import types
import numpy as np
import ml_dtypes
from contextlib import ExitStack


def _install_axon_ntff_hook():
    """Expose the axon NTFF profiling hook under antenv.axon_hooks so
    run_bass_kernel_spmd(trace=True) works in this container. Harmless
    no-op when the hook or .so is unavailable."""
    if 'antenv.axon_hooks' in sys.modules:
        return
    try:
        import antenv
    except ImportError:
        return
    hook = None
    try:
        from trn_agent_boot.trn_boot import _ntff_profile_via_ctypes
        hook = _ntff_profile_via_ctypes('/opt/axon/libaxon_pjrt.so')
    except Exception:
        hook = None
    mod = types.ModuleType('antenv.axon_hooks')
    mod._hook = hook
    mod.get_axon_ntff_profile_hook = lambda: mod._hook
    mod.set_axon_ntff_profile_hook = lambda h: setattr(mod, '_hook', h)
    antenv.axon_hooks = mod
    sys.modules['antenv.axon_hooks'] = mod


_install_axon_ntff_hook()

import concourse.bass as bass
import concourse.tile as tile
import concourse.mybir as mybir
import bass_rust as _br
from concourse import bass_utils


def fix_sync_waits(nc, max_waits=1):
    """Split instructions with >max_waits sem waits: excess waits move to
    preceding same-engine InstNoOp instructions (this walrus build rejects
    multi-wait instructions)."""
    n_split = 0
    uid = 0
    for f in nc.m.functions:
        for bb in f.blocks:
            newl = []
            dirty = False
            for inst in bb.instructions:
                si = inst.sync_info
                if si is not None:
                    waits = list(si.on_wait or [])
                    if len(waits) > max_waits:
                        n_split += 1
                        dirty = True
                        excess = waits[:-max_waits]
                        keep = waits[-max_waits:]
                        for i in range(0, len(excess), max_waits):
                            nop = mybir.InstNoOp(name=f"waitnop{uid}", ins=[], outs=[])
                            uid += 1
                            nop.engine = inst.engine
                            nop.sync_info = _br.SyncInfo(
                                on_wait=excess[i:i+max_waits], on_update=[])
                            newl.append(nop)
                        inst.sync_info = _br.SyncInfo(
                            on_wait=keep, on_update=list(si.on_update or []))
                newl.append(inst)
            if dirty:
                bb.instructions = newl
    return n_split


F = 512; H = 512; C = 100; D = 4
NODES = 15; LEAVES = 16; NBLOCKS = 2
BS = 1024          # per-core batch shard
NT = BS // 128     # b-tiles per core
KT = H // 128      # contraction tiles
EPS = 1e-5

f32 = mybir.dt.float32
f32r = mybir.dt.float32r
bf16 = mybir.dt.bfloat16
f16 = mybir.dt.float16
f8e4 = mybir.dt.float8e4
DR = mybir.MatmulPerfMode.DoubleRow
AF = mybir.ActivationFunctionType
ALU = mybir.AluOpType

# fp8 head-matmul scaling: h is quantized as h*H_SCALE (relu output, max
# ~2.3 -> ~74, well under TRN e4m3 max 240); head weights as w*W_SCALE
# (sigma 0.02*128 = 2.6, lifts most weights out of the subnormal range).
# The psum then carries (H_SCALE*W_SCALE)*logit; INV_SCALE undoes it.
H_SCALE = 32.0
W_SCALE = 128.0
INV_SCALE = 1.0 / (H_SCALE * W_SCALE)


def build_nc(skip_blocks, has_bfi, has_bfs, has_blc,
             fi_dt=f32r, fs_dt=f32r, lnc_dt=f32r, p_dt=f32, trunk_dt=f32r):
    nc = bass.Bass("TRN2", target_bir_lowering=False, debug=False, num_devices=1)
    d = {}
    def din(name, shape, dt):
        d[name] = nc.dram_tensor(name, shape, dt, kind="ExternalInput").ap()
    dr_heads_cfg = (fi_dt == f8e4)
    fast_q = dr_heads_cfg and not has_bfs
    din("xT", [F, BS], trunk_dt)
    if fast_q:
        # x folded into the fs psum via an identity matmul: xn = -SCALE*x
        din("xn", [BS, F], f16)
        din("ident", [128, 128], f16)
    else:
        din("x", [BS, F], f32)
    din("W0", [F, H], trunk_dt)
    din("c0", [H, 1], f32)
    if not skip_blocks:
        for i in range(NBLOCKS):
            din(f"W1_{i}", [H, H], trunk_dt); din(f"c1_{i}", [H, 1], f32)
            din(f"W2_{i}", [H, H], trunk_dt); din(f"c2_{i}", [H, 1], f32)
    din("Wfi", [H, NODES * F], fi_dt)
    din("Wfs", [H, NODES * F], fs_dt)
    din("Wlc", [H, LEAVES * C], lnc_dt)
    if has_bfi: din("bfi", [128, NODES * F], f32)
    if has_bfs: din("bfs", [128, NODES * F], f32)
    if has_blc: din("blc", [128, LEAVES * C], f32)
    y_ap = nc.dram_tensor("y", [BS, C], f32, kind="ExternalOutput").ap()

    with tile.TileContext(nc) as tc, ExitStack() as ctx:
        per = ctx.enter_context(tc.tile_pool(name="per", bufs=1))
        p2ps = ctx.enter_context(tc.tile_pool(name="p2ps", bufs=3, space="PSUM"))
        p3ps = ctx.enter_context(tc.tile_pool(name="p3ps", bufs=2, space="PSUM"))

        xT_w = per.tile([128, KT * BS], trunk_dt, name="xT_w")
        xT_t = [xT_w[:, k*BS:(k+1)*BS] for k in range(KT)]

        def load_xT():
            # emitted AFTER the W0 first-chunk DMA so the trunk's first
            # matmul (needing W0 cols 0:128 + xT k0) starts ASAP
            _xv = xT_w[:].rearrange("p (k c) -> p k c", k=KT)
            _sv = d["xT"].rearrange("(k p) c -> p k c", k=KT)
            for _k in range(KT):
                nc.sync.dma_start(_xv[:, _k:_k+1], _sv[:, _k:_k+1])
        # ---------------- phase 1: trunk ----------------
        def load_w(pool, name, ap, cols, dt=None):
            # two 3D-AP DMAs split by COLUMNS (small first chunk): the m=0
            # matmuls only need the first column block of every k-slab, so
            # compute can start ~4x earlier than with a k-split.
            dt = trunk_dt if dt is None else dt
            wide = pool.tile([128, KT * cols], dt, name=name, tag=name)
            wv = wide[:].rearrange("p (k c) -> p k c", k=KT)
            sv = ap.rearrange("(k p) c -> p k c", k=KT)
            hc = max(cols // 4, 128)
            nc.sync.dma_start(wv[:, :, 0:hc], sv[:, :, 0:hc])
            nc.sync.dma_start(wv[:, :, hc:cols], sv[:, :, hc:cols])
            return [wide[:, k*cols:(k+1)*cols] for k in range(KT)]

        def load_c(pool, name, ap):
            # one merged DMA, triggered from the (startup-idle) gpsimd queue
            # so the Sync engine's ~600ns/trigger budget goes to weights
            w = pool.tile([128, KT], f32, name=name, tag=name)
            nc.gpsimd.dma_start(w[:].rearrange("p (k one) -> p k one", k=KT),
                                ap.rearrange("(k p) one -> p k one", k=KT))
            return [w[:, m:m+1] for m in range(KT)]

        def dense_layerT(p2ps, in_tiles, W_t, c_t, func, out_tiles,
                         fp8_dst=False):
            # out[m][:, bc] = func( sum_k W[k][:,m].T @ in[k][:,bc] + c[m] )
            for m in range(KT):
                for bc in range(BS // 512):
                    ps = p2ps.tile([128, 512], f32, name="fi_ps", tag="fi_ps")
                    for k in range(KT):
                        nc.tensor.matmul(
                            ps[:], W_t[k][:, m*128:(m+1)*128],
                            in_tiles[k][:, bc*512:(bc+1)*512],
                            start=(k == 0), stop=(k == KT - 1))
                    nc.scalar.activation(
                        out_tiles[m][:, bc*512:(bc+1)*512], ps[:], func,
                        bias=c_t[m][:], scale=1.0)
                    if fp8_dst:
                        # fp8 copy straight from psum on the idle DVE:
                        # max(SCALE*ps, 0) == SCALE*relu(ps). Unblocks the
                        # head matmuls ~2.5us before the ACT relu chain ends.
                        nc.vector.tensor_scalar(
                            hT8_w[:, m*BS+bc*512:m*BS+(bc+1)*512], ps[:],
                            H_SCALE, 0.0, op0=ALU.mult, op1=ALU.max)

        hT = [per.tile([128, BS], trunk_dt, name=f"hT{m}") for m in range(KT)]
        dr_heads = (fi_dt == f8e4)
        if dr_heads:
            assert fs_dt == f8e4 and lnc_dt == trunk_dt
        alt = {dt for dt in (fi_dt, fs_dt, lnc_dt) if dt != trunk_dt}
        assert len(alt) <= 1, "at most one non-trunk head dtype supported"
        hb_dt = alt.pop() if alt else None
        if dr_heads:
            # DoubleRow fp8 matmuls need k-tile pairs contiguous in the free
            # dim of ONE tile: [128, (k c)] viewed as [128, k, c]
            hT8_w = per.tile([128, KT * BS], f8e4, name="hT8_w")
            hT8_v = hT8_w[:].rearrange("p (k c) -> p k c", k=KT)
            hTb = None
            need_hb = True
        else:
            hT8_v = None
            hTb = [per.tile([128, BS], hb_dt, name=f"hTb{m}") for m in range(KT)] \
                if hb_dt is not None else None
            need_hb = hTb is not None
        hw_pool = ctx.enter_context(tc.tile_pool(name="hw", bufs=3))
        p2b = ctx.enter_context(tc.tile_pool(name="p2b", bufs=2))

        def _load_head_w(pool, tag, ap, n, dt, eng):
            # single trigger per head: the Sync engine's ~600ns/trigger cost
            # is the scarce resource here, not DMA queue parallelism
            w = pool.tile([128, KT * F], dt, name=tag, tag=tag)
            wv = w[:].rearrange("p (k c) -> p k c", k=KT)
            sv = ap[:, n*F:(n+1)*F].rearrange("(k p) c -> p k c", k=KT)
            eng.dma_start(wv[:, 0:KT], sv[:, 0:KT])
            if dt == f8e4:
                return wv
            return [w[:, k*F:(k+1)*F] for k in range(KT)]

        def load_node_weights(n, eng=None):
            eng = nc.sync if eng is None else eng
            wfi_n = _load_head_w(hw_pool, "wfi_w", d["Wfi"], n, fi_dt, eng)
            wfs_n = _load_head_w(hw_pool, "wfs_w", d["Wfs"], n, fs_dt, eng)
            biases = [None, None]
            if has_bfi:
                ebfi_n = p2b.tile([128, F], f32, name="ebfi_n", tag="ebfi_n")
                eng.dma_start(ebfi_n[:], d["bfi"][:, n*F:(n+1)*F])
                biases[0] = ebfi_n
            if has_bfs:
                bfs_n = p2b.tile([128, F], f32, name="bfs_n", tag="bfs_n")
                eng.dma_start(bfs_n[:], d["bfs"][:, n*F:(n+1)*F])
                biases[1] = bfs_n
            return wfi_n, wfs_n, biases

        p1w = ctx.enter_context(tc.tile_pool(name="p1w", bufs=1))
        if True:
            W0_t = load_w(p1w, "W0t", d["W0"], H)
            load_xT()
            c0_t = load_c(p1w, "c0t", d["c0"])
            h8_inline = dr_heads and skip_blocks
            dense_layerT(p2ps, xT_t, W0_t, c0_t, AF.Relu, hT,
                         fp8_dst=h8_inline)
            if not skip_blocks:
                # rotate block weights through 2 slots; ping-pong activations
                # between o3 and hT so SBUF stays bounded
                with tc.tile_pool(name="blkw", bufs=2) as blkw, \
                     tc.tile_pool(name="blk", bufs=1) as blk:
                    o1 = [blk.tile([128, BS], trunk_dt, name=f"o1_{m}") for m in range(KT)]
                    o2 = [blk.tile([128, BS], trunk_dt, name=f"o2_{m}") for m in range(KT)]
                    o3 = [blk.tile([128, BS], trunk_dt, name=f"o3_{m}") for m in range(KT)]
                    cur = hT
                    for i in range(NBLOCKS):
                        W1_t = load_w(blkw, "Wblk", d[f"W1_{i}"], H)
                        c1_t = load_c(p1w, f"c1t{i}", d[f"c1_{i}"])
                        dense_layerT(p2ps, cur, W1_t, c1_t, AF.Gelu, o1)
                        W2_t = load_w(blkw, "Wblk", d[f"W2_{i}"], H)
                        c2_t = load_c(p1w, f"c2t{i}", d[f"c2_{i}"])
                        dense_layerT(p2ps, o1, W2_t, c2_t, AF.Gelu, o2)
                        nxt = hT if cur is o3 else o3
                        for m in range(KT):
                            nc.vector.tensor_add(nxt[m][:], cur[m][:], o2[m][:])
                        cur = nxt
                    if cur is not hT:
                        for m in range(KT):
                            nc.vector.tensor_copy(hT[m][:], cur[m][:])
        if need_hb and not (dr_heads and skip_blocks):
            for m in range(KT):
                for bc in range(BS // 512):
                    if dr_heads:
                        nc.vector.tensor_scalar(
                            hT8_w[:, m*BS+bc*512:m*BS+(bc+1)*512],
                            hT[m][:, bc*512:(bc+1)*512], H_SCALE, None,
                            op0=ALU.mult)
                    else:
                        nc.vector.tensor_copy(hTb[m][:, bc*512:(bc+1)*512],
                                              hT[m][:, bc*512:(bc+1)*512])
        hT_fi = hTb if (fi_dt != trunk_dt and not dr_heads) else hT
        hT_fs = hTb if (fs_dt != trunk_dt and not dr_heads) else hT
        hT_lc = hTb if lnc_dt != trunk_dt else hT

        # ---------------- phase 2: fi/fs heads + node soft decisions ------
        # phase-3 leaf-logit matmuls are interleaved into the node loop
        # (they only need hT + Wlc); each tile's finalization (sigmoid,
        # routing, weighted leaf sum, output DMA) is emitted right after
        # its last-node work so nothing piles up at the kernel tail.
        if fast_q:
            # xn DMA split: the first group's tiles load before the node-0
            # weights (tiny), the rest after, so head weights win the queue
            xn_w = per.tile([128, NT * F], f16, name="xn_w")
            xn_v = xn_w[:].rearrange("p (t c) -> p t c", t=NT)
            xn_sv = d["xn"].rearrange("(t p) c -> p t c", t=NT)
            nc.gpsimd.dma_start(xn_v[:, 0:3], xn_sv[:, 0:3])
            xn_t = [xn_w[:, t*F:(t+1)*F] for t in range(NT)]
            ident_sb = per.tile([128, 128], f16, name="ident_sb")
            nc.gpsimd.dma_start(ident_sb[:], d["ident"][:])
            x_t = None
        else:
            x_w = per.tile([128, NT * F], f32, name="x_w")
            nc.sync.dma_start(x_w[:].rearrange("p (t c) -> p t c", t=NT),
                              d["x"].rearrange("(t p) c -> p t c", t=NT))
            x_t = [x_w[:, t*F:(t+1)*F] for t in range(NT)]
        stats = ctx.enter_context(tc.tile_pool(name="stats", bufs=1))
        den_t = [stats.tile([128, NODES], f32, name=f"den{t}") for t in range(NT)]
        num_t = [stats.tile([128, NODES], f32, name=f"num{t}") for t in range(NT)]
        lsb_dt = f16 if dr_heads_cfg else f32
        lsb_t = [stats.tile([128, LEAVES * C], lsb_dt, name=f"lsb{t}")
                 for t in range(NT)]

        p2sc = ctx.enter_context(tc.tile_pool(name="p2sc", bufs=3))
        lcw = ctx.enter_context(tc.tile_pool(name="lcw", bufs=1))
        small = ctx.enter_context(tc.tile_pool(name="smal", bufs=2))
        p3sc = ctx.enter_context(tc.tile_pool(name="p3sc", bufs=2))
        p3b = ctx.enter_context(tc.tile_pool(name="p3b", bufs=1))
        wlc_holder = [None, None]

        CC = 4 * C    # 400-col psum chunks (4 leaves, one PSUM bank)
        NCH = (LEAVES * C) // CC

        def emit_lnc(t):
            # leaf logits for tile t -> lsb_t[t] (SBUF, fp32)
            wlc_t, blc_sb = wlc_holder
            for ci in range(NCH):
                cols = slice(ci * CC, (ci + 1) * CC)
                lps = p3ps.tile([128, CC], f32, name="lps", tag="lps")
                for k in range(KT):
                    nc.tensor.matmul(lps[:], hT_lc[k][:, t*128:(t+1)*128],
                                     wlc_t[k][:, cols],
                                     start=(k == 0), stop=(k == KT-1))
                if has_blc:
                    nc.vector.tensor_tensor(lsb_t[t][:, cols], lps[:],
                                            blc_sb[:, cols], op=ALU.add)
                elif ci % 2 == 0:
                    # split the psum->sbuf copies between ACT and DVE so
                    # neither engine eats the whole 19us
                    nc.scalar.copy(lsb_t[t][:, cols], lps[:])
                else:
                    nc.vector.tensor_copy(lsb_t[t][:, cols], lps[:])

        def finalize_tile(t, tail=False):
            with nc.allow_low_precision(
                    reason="sd/coeff routing products are bounded [0,1]; "
                           "f16 keeps the DVE at 2x rate"):
                _finalize_tile(t, tail)

        def _finalize_tile(t, tail):
            # sd = sigmoid(num/den), via Exp (stays on the already-loaded ACT
            # Exp table; a Sigmoid table swap costs ~1.3us per load)
            rden = small.tile([128, NODES], f32, name="rden", tag="rden")
            nc.vector.reciprocal(rden[:], den_t[t][:])
            ratio = small.tile([128, NODES], f32, name="ratio", tag="ratio")
            nc.vector.tensor_tensor(ratio[:], num_t[t][:], rden[:], op=ALU.mult)
            er = small.tile([128, NODES], f32, name="er", tag="er")
            nc.scalar.activation(er[:], ratio[:], AF.Exp, scale=-1.0)
            er1 = small.tile([128, NODES], f32, name="er1", tag="er1")
            nc.vector.tensor_scalar(er1[:], er[:], 1.0, None, op0=ALU.add)
            # sd/nsd interleaved in one tile (col 2j = sd_j, col 2j+1 =
            # 1-sd_j) so the routing products read node pairs as strided
            # APs directly - no gpsimd interleave copies needed
            sdn = small.tile([128, 2 * NODES], f16, name="sdn", tag="sdn")
            sdnv = sdn[:].rearrange("p (n two) -> p n two", two=2)
            nc.vector.reciprocal(sdnv[:, :, 0:1], er1[:].unsqueeze(2))
            nc.vector.tensor_scalar(sdnv[:, :, 1:2], sdnv[:, :, 0:1], -1.0,
                                    1.0, op0=ALU.mult, op1=ALU.add)
            # tree routing: coeff[leaf] = prod_d (sd | 1-sd) down the path
            coeff = small.tile([128, LEAVES], f16, name="coeff", tag="coeff")
            cur = sdn[:, 0:2]
            off = 1
            for dlev in range(2, D + 1):
                w = 1 << dlev
                out = coeff if dlev == D else small.tile(
                    [128, w], f16, name=f"c{dlev}", tag=f"c{dlev}")
                rep = cur.unsqueeze(2).broadcast_to([128, w // 2, 2])
                ov = out[:].rearrange("p (a two) -> p a two", two=2)
                iv = sdnv[:, off:off + w // 2, :]
                nc.vector.tensor_mul(ov, rep, iv)
                cur = out[:]
                off += w // 2
            # weighted leaf sum: q3 = lsb * coeff (leaf-broadcast). Wlc
            # columns are host-permuted to class-major (c*LEAVES+l), so the
            # leaf-weighted sum is a contiguous inner-dim reduce; 16-bit
            # operands run the DVE at 2x
            q3 = p3sc.tile([128, LEAVES * C], lsb_dt, name="q3", tag=f"q3{t%2}")
            q3v = q3[:].rearrange("p (c l) -> p c l", l=LEAVES)
            lv = lsb_t[t][:].rearrange("p (c l) -> p c l", l=LEAVES)
            cv = coeff[:].unsqueeze(1).broadcast_to([128, C, LEAVES])
            nc.vector.tensor_mul(q3v, lv, cv)
            outt = p3sc.tile([128, C], f32, name="outt", tag=f"outt{t%2}")
            # leaf reduce as a pairwise f16 add tree. Tail tiles run it on
            # DVE (lower per-op overhead -> shortest chain latency);
            # mid-stream tiles on the idle gpsimd so DVE stays free for q.
            eng = nc.vector if tail else nc.gpsimd
            cur_v, w = q3v, LEAVES
            while w > 2:
                half = p3sc.tile([128, C * (w // 2)], lsb_dt,
                                 name=f"lred{w}", tag=f"lred{w}_{t%2}")
                hv = half[:].rearrange("p (c l) -> p c l", l=w // 2)
                eng.tensor_add(hv, cur_v[:, :, 0:w//2], cur_v[:, :, w//2:w])
                cur_v, w = hv, w // 2
            eng.tensor_add(outt[:].rearrange("p (c l) -> p c l", l=1),
                           cur_v[:, :, 0:1], cur_v[:, :, 1:2])
            nc.sync.dma_start(y_ap[t*128:(t+1)*128, :], outt[:])

        lg_scale = INV_SCALE if dr_heads else 1.0

        def node_tile_body(n, t, wfi_n, wfs_n, ebfi_n, bfs_n):
            if dr_heads:
                # fp8 DoubleRow: virtual 128x256 array, 2 k-tiles per matmul.
                # fi/fs interleaved so each hT8 stationary pair is loaded
                # once and used by two consecutive matmuls.
                fi_ps = p2ps.tile([128, F], f32, name="fi_ps", tag="fi_ps")
                fs_ps = p2ps.tile([128, F], f32, name="fs_ps", tag="fs_ps")
                nkp = KT // 2
                for kp in range(nkp):
                    hsl = hT8_v[:, 2*kp:2*kp+2, t*128:(t+1)*128]
                    nc.tensor.matmul(fi_ps[:], hsl, wfi_n[:, 2*kp:2*kp+2, :],
                                     start=(kp == 0), stop=(kp == nkp - 1),
                                     perf_mode=DR)
                    nc.tensor.matmul(fs_ps[:], hsl, wfs_n[:, 2*kp:2*kp+2, :],
                                     start=(kp == 0),
                                     stop=(kp == nkp - 1) and not fast_q,
                                     perf_mode=DR)
                if fast_q:
                    # fold -SCALE*x into the fs psum: identity stationary,
                    # xn moving -> psum += -SCALE*x. Kills the separate
                    # (x - fs) DVE pass entirely.
                    nc.tensor.matmul(fs_ps[:], ident_sb[:], xn_t[t][:],
                                     start=False, stop=True)
            else:
                fi_ps = p2ps.tile([128, F], f32, name="fi_ps", tag="fi_ps")
                for k in range(KT):
                    nc.tensor.matmul(fi_ps[:], hT_fi[k][:, t*128:(t+1)*128],
                                     wfi_n[k][:], start=(k == 0), stop=(k == KT-1))
                fs_ps = p2ps.tile([128, F], f32, name="fs_ps", tag="fs_ps")
                for k in range(KT):
                    nc.tensor.matmul(fs_ps[:], hT_fs[k][:, t*128:(t+1)*128],
                                     wfs_n[k][:], start=(k == 0), stop=(k == KT-1))
            P = p2sc.tile([128, F], p_dt, name="P", tag="P")
            if has_bfi:
                P0 = p2sc.tile([128, F], f32, name="P0", tag="P0")
                nc.scalar.activation(P0[:], fi_ps[:], AF.Exp, scale=lg_scale)
                nc.vector.tensor_tensor(
                    P[:], P0[:], ebfi_n[:], op=ALU.mult)
                qd = p2sc.tile([128, F], f32, name="qd", tag="qd")
                nc.vector.tensor_scalar(
                    qd[:], P[:], 1.0, 0.0, op0=ALU.mult, op1=ALU.add,
                    accum_out=den_t[t][:, n:n+1])
            else:
                nc.scalar.activation(P[:], fi_ps[:], AF.Exp, scale=lg_scale,
                                     accum_out=den_t[t][:, n:n+1])
            q = p2sc.tile([128, F], bf16, name="q", tag="q")
            if fast_q:
                # fs_ps already holds SCALE*(fs - x); one fused DVE op:
                # q = (-INV*fs_ps) * P = P*(x - fs), accumulated into num
                nc.vector.scalar_tensor_tensor(
                    q[:], fs_ps[:], -lg_scale, P[:], op0=ALU.mult,
                    op1=ALU.mult, accum_out=num_t[t][:, n:n+1])
            else:
                tdiff = p2sc.tile([128, F], p_dt, name="tdiff", tag="tdiff")
                if has_bfs:
                    t0 = p2sc.tile([128, F], f32, name="t0", tag="t0")
                    nc.vector.scalar_tensor_tensor(
                        t0[:], fs_ps[:], -lg_scale, x_t[t][:],
                        op0=ALU.mult, op1=ALU.add)
                    nc.vector.tensor_tensor(
                        tdiff[:], t0[:], bfs_n[:], op=ALU.subtract)
                else:
                    nc.vector.scalar_tensor_tensor(
                        tdiff[:], fs_ps[:], -lg_scale, x_t[t][:],
                        op0=ALU.mult, op1=ALU.add)
                nc.vector.scalar_tensor_tensor(
                    q[:], P[:], 1.0, tdiff[:], op0=ALU.mult, op1=ALU.mult,
                    accum_out=num_t[t][:, n:n+1])

        # phase 2 in three tile-groups: earlier groups' finalization chains
        # run while later groups are still doing matmuls, so only the last
        # (smallest) group's chains can stack up at the kernel tail. Costs
        # extra passes over the fi/fs weights (fp8: DMA hidden under compute).
        GROUPS = [(0, 1, 2), (3, 4, 5), (6, 7)]
        for g, tiles in enumerate(GROUPS):
            for n in range(NODES):
                wfi_n, wfs_n, (ebfi_n, bfs_n) = load_node_weights(n)
                if fast_q and g == 0 and n == 0:
                    # rest of xn: consumers are in groups 1-2, emitted later
                    nc.sync.dma_start(xn_v[:, 3:NT], xn_sv[:, 3:NT])
                if g == 0 and n == 3:
                    wlc_holder[0] = load_w(lcw, "wlc", d["Wlc"], LEAVES * C,
                                           dt=lnc_dt)
                    if has_blc:
                        bt = p3b.tile([128, LEAVES * C], f32, name="blc_sb")
                        nc.sync.dma_start(bt[:], d["blc"][:])
                        wlc_holder[1] = bt
                for t in tiles:
                    node_tile_body(n, t, wfi_n, wfs_n, ebfi_n, bfs_n)
                    if n == NODES - 1:
                        finalize_tile(t, tail=(g == len(GROUPS) - 1))
                if n >= 5 and (n - 5) % 3 == 0 and (n - 5) // 3 < len(tiles):
                    emit_lnc(tiles[(n - 5) // 3])

    return nc


def prep_core_inputs(core, x, w_in, b_in, g0, be0, bw1, bb1, bg1, bbe1,
                     bw2, bb2, bg2, bbe2, w_fi, b_fi, w_fs, b_fs, w_lc, b_lc,
                     skip_blocks, fi_np=np.float32, fs_np=np.float32,
                     lnc_np=np.float32, trunk_np=np.float32):
    """Host-side prep: shard x, fold BN into weights, transpose x."""
    r = 1.0 / np.sqrt(np.float32(1.0) + np.float32(EPS))
    xs = np.ascontiguousarray(x[core*BS:(core+1)*BS]).astype(np.float32)

    def head_cast(w, np_dt):
        if np_dt is ml_dtypes.float8_e4m3:
            # scale weights out of the fp8 subnormal range; clip to TRN
            # e4m3 max normal (+-240). Compensated on-device via INV_SCALE.
            return np.ascontiguousarray(
                np.clip(w * W_SCALE, -240.0, 240.0)).astype(np_dt)
        return np.ascontiguousarray(w).astype(np_dt)

    fastq = (fi_np is ml_dtypes.float8_e4m3) and not np.any(b_fs)
    m = {
        "xT": np.ascontiguousarray(xs.T).astype(trunk_np),
        "W0": np.ascontiguousarray(w_in * (g0 * r)[None, :]).astype(trunk_np),
        "c0": (b_in * g0 * r + be0).astype(np.float32).reshape(H, 1),
        "Wfi": head_cast(w_fi, fi_np),
        "Wfs": head_cast(w_fs, fs_np),
        # class-major leaf-inner layout: col (l*C + c) -> (c*LEAVES + l)
        "Wlc": np.ascontiguousarray(
            w_lc.reshape(H, LEAVES, C).transpose(0, 2, 1).reshape(
                H, LEAVES * C)).astype(lnc_np),
    }
    if fastq:
        m["xn"] = np.ascontiguousarray(
            (-(H_SCALE * W_SCALE)) * xs).astype(np.float16)
        m["ident"] = np.eye(128, dtype=np.float16)
    else:
        m["x"] = xs
    if not skip_blocks:
        for i in range(NBLOCKS):
            s1 = bg1[i] * r
            m[f"W1_{i}"] = np.ascontiguousarray(bw1[i] * s1[None, :]).astype(trunk_np)
            m[f"c1_{i}"] = (bb1[i] * s1 + bbe1[i]).astype(np.float32).reshape(H, 1)
            s2 = bg2[i] * r
            m[f"W2_{i}"] = np.ascontiguousarray(bw2[i] * s2[None, :]).astype(trunk_np)
            m[f"c2_{i}"] = (bb2[i] * s2 + bbe2[i]).astype(np.float32).reshape(H, 1)
    if np.any(b_fi):
        m["bfi"] = np.ascontiguousarray(np.broadcast_to(
            np.exp(b_fi.astype(np.float32))[None, :], (128, NODES * F)))
    if np.any(b_fs):
        m["bfs"] = np.ascontiguousarray(np.broadcast_to(
            b_fs.astype(np.float32)[None, :], (128, NODES * F)))
    if np.any(b_lc):
        blc_cm = b_lc.astype(np.float32).reshape(LEAVES, C).T.reshape(-1)
        m["blc"] = np.ascontiguousarray(np.broadcast_to(
            blc_cm[None, :], (128, LEAVES * C)))
    return m


N_CORES = 8
B_FULL = 8192

# populated by kernel() when BASS_TRACE=1 (NTFF profiling enabled)
last_exec_time_ns = None
last_trace_path = None


def _reference_numpy(x, w_in, b_in, g0, be0, bw1, bb1, bg1, bbe1,
                     bw2, bb2, bg2, bbe2, w_fi, b_fi, w_fs, b_fs,
                     w_lc, b_lc):
    """Pure-numpy fallback for shapes this kernel was not compiled for."""
    from scipy.special import erf
    def bn(h, g, b):
        return h * (g / np.sqrt(1.0 + EPS)) + b
    def gelu(v):
        return v * 0.5 * (1.0 + erf(v / np.sqrt(2.0)))
    xx = x.astype(np.float64)
    h = np.maximum(bn(xx @ w_in.astype(np.float64) + b_in, g0, be0), 0.0)
    for i in range(bw1.shape[0]):
        r = h
        o = gelu(bn(h @ bw1[i].astype(np.float64) + bb1[i], bg1[i], bbe1[i]))
        o = gelu(bn(o @ bw2[i].astype(np.float64) + bb2[i], bg2[i], bbe2[i]))
        h = o + r
    Bn = xx.shape[0]
    nodes = w_fi.shape[1] // x.shape[1]
    leaves = nodes + 1
    ncls = w_lc.shape[1] // leaves
    fi = (h @ w_fi.astype(np.float64) + b_fi).reshape(Bn, nodes, -1)
    fs = (h @ w_fs.astype(np.float64) + b_fs).reshape(Bn, nodes, -1)
    lnc = (h @ w_lc.astype(np.float64) + b_lc).reshape(Bn, leaves, ncls)
    e = np.exp(fi - fi.max(-1, keepdims=True))
    sfi = e / e.sum(-1, keepdims=True)
    sd = 1.0 / (1.0 + np.exp(-(np.einsum('bnf,bf->bn', sfi, xx)
                               - (sfi * fs).sum(-1))))
    depth = int(np.log2(leaves))
    coeff = np.ones((Bn, leaves))
    for l in range(leaves):
        for dd in range(1, depth + 1):
            node = (2 ** (dd - 1) * (2 ** depth + l) - 2 ** depth) // (2 ** depth)
            side = (l // 2 ** (depth - dd)) % 2
            p = sd[:, node]
            coeff[:, l] *= p if side == 0 else (1.0 - p)
    return np.einsum('bl,blc->bc', coeff, lnc).astype(np.float32)


def kernel(x, w_in, b_in, g0, be0, bw1, bb1, bg1, bbe1, bw2, bb2, bg2, bbe2,
           w_fi, b_fi, w_fs, b_fs, w_lc, b_lc):
    global last_exec_time_ns, last_trace_path
    inputs = dict(x=x, w_in=w_in, b_in=b_in, g0=g0, be0=be0, bw1=bw1,
                  bb1=bb1, bg1=bg1, bbe1=bbe1, bw2=bw2, bb2=bb2, bg2=bg2,
                  bbe2=bbe2, w_fi=w_fi, b_fi=b_fi, w_fs=w_fs, b_fs=b_fs,
                  w_lc=w_lc, b_lc=b_lc)
    inputs = {k: np.asarray(v) for k, v in inputs.items()}
    x = inputs["x"]
    if (x.shape != (B_FULL, F) or inputs["w_in"].shape != (F, H)
            or inputs["w_fi"].shape != (H, NODES * F)
            or inputs["w_lc"].shape != (H, LEAVES * C)
            or inputs["bw1"].shape != (NBLOCKS, H, H)):
        return _reference_numpy(**inputs)

    # residual blocks are exactly identity when the second BN has zero
    # weight and bias (gelu(0) == 0); detected from the actual values
    skip_blocks = (not np.any(inputs["bg2"])) and (not np.any(inputs["bbe2"]))
    has_bfi = bool(np.any(inputs["b_fi"]))
    has_bfs = bool(np.any(inputs["b_fs"]))
    has_blc = bool(np.any(inputs["b_lc"]))

    nc = build_nc(skip_blocks, has_bfi, has_bfs, has_blc,
                  fi_dt=f8e4, fs_dt=f8e4, lnc_dt=f16, p_dt=f16, trunk_dt=f16)
    fix_sync_waits(nc, max_waits=1)

    in_maps = [prep_core_inputs(c, **inputs, skip_blocks=skip_blocks,
                                fi_np=ml_dtypes.float8_e4m3,
                                fs_np=ml_dtypes.float8_e4m3,
                                lnc_np=np.float16, trunk_np=np.float16)
               for c in range(N_CORES)]
    # the axon-tunneled device occasionally reports a transient
    # NRT_EXEC_UNIT_UNRECOVERABLE on execute; retry a couple of times
    import time as _time
    res = None
    for attempt in range(3):
        try:
            res = bass_utils.run_bass_kernel_spmd(
                nc, in_maps, core_ids=list(range(N_CORES)))
            break
        except Exception:
            if attempt == 2:
                raise
            _time.sleep(10.0)
    last_exec_time_ns = res.exec_time_ns
    last_trace_path = (res.instructions_and_trace[1]
                       if res.instructions_and_trace else None)
    return np.concatenate([res.results[c]["y"] for c in range(N_CORES)],
                          axis=0).astype(np.float32, copy=False)

